# revision 60
# baseline (speedup 1.0000x reference)
"""Binarized conv1d (k=7, pad=3 with -1.0) + maxpool(2) + PReLU + BatchNorm1d
(training stats) fused Trainium2 kernel, data-parallel over batch N across 8
NeuronCores with an on-chip AllReduce for the BN batch statistics.

Contract: kernel(**inputs) takes the FULL inputs from setup_inputs() and
returns the FULL [128, 128, 2048] float32 output.

Layout / algorithm per core (16 of the 128 batches):
  - conv as 2 accumulated fp8e4 DoubleRow matmuls per 512-wide PSUM tile:
    each matmul contracts K=256 = 128 partitions (64 in-channels x 2 tap
    groups) x 2 fp8 elements per PE cell. The two per-cell elements are
    adjacent tap planes, addressed as an overlapping 3D access pattern
    [128, 2, 512] with element stride 2 into the same activation buffer, so
    the DoubleRow pairing costs no extra data movement. sign(x) in fp8e4 is
    exact (+-1) and conv sums are even integers |.|<=448, so PSUM f32
    results are bit-exact.
  - the activation buffer holds sign(x_padded) in one 64-partition half and
    the same data shifted one position in the other half. One full-width
    128-partition Sign per batch-pair (per-lane engine: same cost as 64
    rows) lands in the even batch's buffer; three partition-crossing SBUF
    DMAs derive the odd batch's buffer and the shifted halves. Even/odd
    batches use host-precomputed swapped-half weight layouts.
  - maxpool k2s2 + PReLU, engine-balanced per half-tile: scheme A pools on
    the vector engine straight out of PSUM (reduce_max) then PReLUs on the
    scalar engine (accum_out = per-channel sum); scheme B PReLUs on the
    scalar engine straight out of PSUM (prelu is monotone, so
    prelu(max)=max(prelu)) then pools on the vector engine with a 2-port
    strided f16 max (accum_out = sum). Each batch drains one PSUM tile via
    each engine; B_HALVES tunes the split.
  - per-channel sum(y^2) via one scalar_tensor_tensor y*y per batch,
    deferred one batch so it never delays a PSUM-draining pool.
  - partial (sum, sumsq) all-reduced across the 8 cores with direct
    remote-DMA SBUF broadcasts (XOR-slot exchange). Descriptor generation
    runs early on the idle gpsimd engine; the deferred source read means
    only trigger_dma + seven 1KB transfers follow the local stats. A
    prelude-AllGather barrier keeps remote writes after every peer's
    semaphore init.
  - scale/shift from the global stats (one Newton step on sqrt), then the
    normalization streams back to HBM as f16 (halves store traffic; the
    host widens to f32; ~2e-4 rel err, well within the 2e-2 gate).
  - x loads, at-copies and stores share the sync/scalar HWDGE queues in an
    order that keeps the latency-critical at-copies (which gate the
    matmuls) ahead of the next 6us x transfer.
"""

import uuid

import numpy as np
import ml_dtypes
import jax
import bass_rust as _br

# The jax persistent compilation cache mis-keys bass_exec custom-call
# executables (the embedded NEFF differs while the cache key does not),
# which can hand back a stale executable and wedge the device. Disable it.
jax.config.update("jax_enable_compilation_cache", False)

import concourse.bacc as bacc
import concourse.mybir as mybir
import concourse.tile as tile
from concourse.bass_utils import run_bass_kernel_spmd

AF = mybir.ActivationFunctionType
ALU = mybir.AluOpType

N_CORES = 8
N = 128            # total batch
NB = N // N_CORES  # batches per core = 16
CI = 64            # in channels
CO = 128           # out channels
L = 4096           # input length
LP = L + 7         # padded width: cols 0-2 pad, 3..4098 data, 4099-4102 pad
LO = L // 2        # pooled output length = 2048
K = 7              # kernel taps
PAD_VAL = -1.0
EPS = 1e-5
M_GLOBAL = float(N * LO)  # BN reduction count per channel

XT_BUFS = 3
AT_BUFS = 4
CS = 2056          # ramp column split: covers the first PSUM tile's reads

# Engine load-balance knob (tuned against perfetto engine-busy numbers):
# half-tiles (index 2*b+half) in B_HALVES run PReLU on the scalar engine
# directly on PSUM (prelu is monotone, so prelu(max)=max(prelu)) and pool on
# the vector engine via a 2-port SBUF max; the rest pool on vector straight
# from PSUM and PReLU on scalar. One B half + one A half per batch means the
# two PSUM tiles of a batch retire on different engines in parallel; the
# extra B halves even out the remaining vector-vs-scalar busy gap.
B_HALVES = frozenset([2 * b for b in range(16)]
                     + [2 * b + 1 for b in (1, 3, 7, 9, 11, 14)])


def _build(alpha: float, repeats: int = 1, copy_hwdge: bool = True):
    nc = bacc.Bacc("TRN2", target_bir_lowering=False, debug=False,
                   num_devices=N_CORES)

    xs = nc.dram_tensor("xs", [NB * CI, L], mybir.dt.float32, kind="ExternalInput")
    wts = nc.dram_tensor("wts", [128, 8 * 128], mybir.dt.float8e4, kind="ExternalInput")
    gb = nc.dram_tensor("gb", [128, 2], mybir.dt.float32, kind="ExternalInput")
    out = nc.dram_tensor("out", [NB * CO, LO], mybir.dt.float16, kind="ExternalOutput")

    with tile.TileContext(nc) as tc:
        with (
            tc.tile_pool(name="wp", bufs=1) as wp,
            tc.tile_pool(name="xp", bufs=XT_BUFS) as xp,
            tc.tile_pool(name="ap", bufs=AT_BUFS) as ap_pool,
            tc.tile_pool(name="pp", bufs=2, space="PSUM") as pp,
            tc.tile_pool(name="mp", bufs=4) as mp,
            tc.tile_pool(name="yp", bufs=NB) as yp,
            tc.tile_pool(name="sp", bufs=1) as sp,
            tc.tile_pool(name="qp", bufs=2) as qp,
            tc.tile_pool(name="op", bufs=3) as op_pool,
            tc.tile_pool(name="dp", bufs=1, space="DRAM") as dp,
        ):
            # weights/params go via the scalar-engine HWDGE queue so the
            # first big x load owns the sync queue from t=0
            wt = wp.tile([128, 8 * 128], mybir.dt.float8e4)
            nc.scalar.dma_start(wt[:], wts[:])
            gbt = wp.tile([128, 2], mybir.dt.float32)
            nc.scalar.dma_start(gbt[:], gb[:])

            # trigger the ACT table load during the DMA ramp, off-path
            warm = wp.tile([128, 1], mybir.dt.float32)
            nc.vector.memset(warm[:], 1.0)
            nc.scalar.activation(warm[:], warm[:], AF.Sign)

            for _rep in range(repeats):
                _build_pass(nc, tc, xs, out, wt, gbt, alpha,
                            xp, ap_pool, pp, mp, yp, sp, qp, op_pool, dp, _rep,
                            copy_hwdge)

    nc.compile()
    nc.m.name = f"bk{uuid.uuid4().hex[:10]}"
    return nc


def _build_pass(nc, tc, xs, out, wt, gbt, alpha,
                xp, ap_pool, pp, mp, yp, sp, qp, op_pool, dp, rep,
                copy_hwdge=True):
    # stats: cols 0:32 per-half-tile sum accums, 32:48 per-batch sumsq
    stats = sp.tile([128, 48], mybir.dt.float32, name="stats", tag="stats")

    # all-reduce buffers (persistent pool: stable addresses, single-writer
    # slots, so remote writes may land any time after the barrier)
    loc = sp.tile([128, 2], mybir.dt.float32, name="loc", tag="loc")
    xbuf = sp.tile([128, 16], mybir.dt.float32, name="xbuf", tag="xbuf")
    g = sp.tile([128, 2], mybir.dt.float32, name="g", tag="g")
    rsem = nc.alloc_semaphore("ar_remote")
    lsem = nc.alloc_semaphore("ar_local")

    def _ar_descgen():
        # Each core broadcasts its [128,2] partial (sum, sumsq) into peer
        # SBUFs: for delta in 1..7, core c sends to core c^delta, landing in
        # xbuf slot delta; the XOR pairing makes every slot single-writer.
        # The SWDGE descriptor generation (~7us serial on gpsimd) runs here,
        # during pass 1 while gpsimd idles (but after the bp0 pad memsets so
        # it doesn't delay the first matmul); the descriptors' source read of
        # `loc` is deferred to trigger time, so only trigger_dma + the 1KB
        # transfers sit on the post-stats critical path.
        for delta in range(1, 8):
            rd = [None] * 8
            rd[delta] = (0, delta)
            nc.gpsimd.remote_dma_broadcast(
                xbuf[:, 2 * delta:2 * delta + 2], loc[:, 0:2],
                rsem, lsem, rdests=rd,
            )

    def _dr_rhs(at, col):
        # DoubleRow moving operand: [128, 2 k-tiles, 512] where the 2
        # contraction elements per cell are the activation at col+{0,+2}
        # (adjacent tap-pair planes = column shifts in the same at buffer).
        rhs = at[:, col:col + 512].copy()
        part = rhs.ap[0]
        rhs.ap = _br.VecI64Pair([(part[0], part[1]), (2, 2), (1, 512)])
        return rhs

    def _xt_load(bp):
        xt = xp.tile([128, LP], mybir.dt.float32, name=f"xt{bp}", tag="xt")
        # the x DMA only writes 3:4099; stamp the pad columns every time
        # (cheap on gpsimd, keeps the race checker clean)
        nc.gpsimd.memset(xt[:, 0:3], PAD_VAL)
        nc.gpsimd.memset(xt[:, L + 3:LP], PAD_VAL)
        # everything rides the sync HWDGE queue; emission order guarantees
        # each pair's latency-critical at-copies precede the NEXT pair's big
        # x transfer, so the copies (which gate the matmuls) never queue
        # behind a 6us load
        nc.sync.dma_start(xt[:, 3:L + 3], xs[bp * 128:(bp + 1) * 128, :])
        return xt

    def _xt_load0():
        # Pair 0's load is column-split across TWO pool tiles so the first
        # sign (which Tile dep-tracks per tile, not per column range) fires
        # as soon as the first ~1MB lands rather than after the full 2.1MB.
        xta = xp.tile([128, LP], mybir.dt.float32, name="xt0a", tag="xt")
        xtb = xp.tile([128, LP], mybir.dt.float32, name="xt0b", tag="xt")
        nc.gpsimd.memset(xta[:, 0:3], PAD_VAL)
        nc.gpsimd.memset(xtb[:, L + 3:LP], PAD_VAL)
        nc.sync.dma_start(xta[:, 3:CS], xs[0:128, 0:CS - 3])
        nc.sync.dma_start(xtb[:, CS:L + 3], xs[0:128, CS - 3:L])
        return xta, xtb

    def _pair_tiles(bp):
        at0 = ap_pool.tile([128, LP], mybir.dt.float8e4,
                           name=f"at{2 * bp}", tag="at")
        at1 = ap_pool.tile([128, LP], mybir.dt.float8e4,
                           name=f"at{2 * bp + 1}", tag="at")
        return at0, at1

    def _pair_prep(bp, xt):
        # One full-width Sign per batch-pair (the ACT engine is per-lane
        # throughput, so [128,*] costs the same as [64,*]). It lands in the
        # even batch's tile at0: rows 0:64 are b0's direct half already;
        # rows 64:128 transiently hold b1's direct data, which is copied out
        # to at1 (direct + shifted) before being overwritten by b0's shifted
        # copy. b1's matmuls therefore run first.
        at0, at1 = _pair_tiles(bp)
        nc.scalar.activation(at0[:, :], xt[:, :], AF.Sign)
        nc.sync.dma_start(at1[64:128, :], at0[64:128, :])
        nc.sync.dma_start(at1[0:64, 0:LP - 1], at0[64:128, 1:LP])
        nc.sync.dma_start(at0[64:128, 0:LP - 1], at0[0:64, 1:LP])
        return at0, at1

    def _prep0_chunk(at0, at1, xt, c0, c1):
        # ramp: sign+copies for columns [c0, c1) only, tracking the chunked
        # x load; emitted right before the matmuls that consume them so the
        # Tile queue-count waits don't drag in later transfers
        nc.scalar.activation(at0[:, c0:c1], xt[:, c0:c1], AF.Sign)
        nc.sync.dma_start(at1[64:128, c0:c1], at0[64:128, c0:c1])
        lo = max(c0 - 1, 0)
        nc.sync.dma_start(at1[0:64, lo:c1 - 1], at0[64:128, lo + 1:c1])

    def _prep0_shift(at0):
        nc.sync.dma_start(at0[64:128, 0:LP - 1], at0[0:64, 1:LP])

    y_tiles = [None] * NB
    pending_sq = []

    def _emit_sq():
        b, yt = pending_sq.pop(0)
        sq = qp.tile([128, LO], mybir.dt.bfloat16, name=f"sq{b}", tag="sq")
        nc.vector.scalar_tensor_tensor(
            sq[:], yt[:], 1.0, yt[:],
            op0=ALU.mult, op1=ALU.mult,
            accum_out=stats[:, 32 + b:33 + b],
        )

    xt0a, xt0b = _xt_load0()
    prep_next = _pair_tiles(0)
    _prep0_chunk(prep_next[0], prep_next[1], xt0a, 0, CS)
    for bp in range(NB // 2):
        at0, at1 = prep_next
        xt = xt0b if bp == 0 else xt_next
        # prefetch the next pair's input BEFORE this pair's remaining
        # at-copies enter the sync HWDGE queue
        if bp + 1 < NB // 2:
            xt_next = _xt_load(bp + 1)

        for sub in (1, 0):
            b = 2 * bp + sub
            at = at1 if sub == 1 else at0

            # emit the next pair's sign+copies between this pair's two
            # batches: the ACT engine starts the sign as soon as the x tile
            # lands instead of after this pair's prelus, and the PE never
            # starves waiting for at-tiles
            if sub == 0 and bp + 1 < NB // 2:
                prep_next = _pair_prep(bp + 1, xt_next)

            yt = yp.tile([128, LO], mybir.dt.float16, name=f"yt{b}", tag="yt")
            y_tiles[b] = yt

            for half in range(2):
                h = half * 2048
                ps = pp.tile([128, 2048], mybir.dt.float32,
                             name=f"ps{b}_{half}", tag="ps")
                for pair in range(2):
                    j0 = sub * 4 + 2 * pair
                    w_pair = wt[:, j0 * 128:(j0 + 2) * 128].rearrange(
                        "p (e c) -> p e c", e=2)
                    for t in range(4):
                        nc.tensor.matmul(
                            ps[:, t * 512:(t + 1) * 512],
                            w_pair,
                            _dr_rhs(at, h + t * 512 + 4 * pair),
                            start=(pair == 0), stop=(pair == 1),
                            perf_mode=mybir.MatmulPerfMode.DoubleRow,
                        )
                if bp == 0 and sub == 1 and half == 0:
                    # ramp: second-chunk sign+copies emitted only after the
                    # first-chunk matmuls, so their queue waits don't gate it
                    _prep0_chunk(at0, at1, xt, CS, LP)
                hidx = 2 * b + half
                sum_col = stats[:, hidx:hidx + 1]
                if hidx in B_HALVES:
                    # PReLU straight off PSUM on scalar, then 2-port pooled
                    # max on vector (SBUF f16), accumulating sum(y).
                    pt = mp.tile([128, 2048], mybir.dt.float16,
                                 name=f"pt{hidx}", tag="pt")
                    nc.scalar.activation(pt[:], ps[:], AF.Prelu, alpha=alpha)
                    ptr = pt.rearrange("p (t two) -> p two t", two=2)
                    nc.vector.scalar_tensor_tensor(
                        yt[:, half * 1024:(half + 1) * 1024],
                        ptr[:, 0, :], 1.0, ptr[:, 1, :],
                        op0=ALU.mult, op1=ALU.max,
                        accum_out=sum_col,
                    )
                else:
                    mt = mp.tile([128, 1024], mybir.dt.float16,
                                 name=f"mt{hidx}", tag="mt")
                    nc.vector.tensor_reduce(
                        mt[:],
                        ps.rearrange("p (t two) -> p t two", two=2),
                        axis=mybir.AxisListType.X, op=ALU.max,
                    )
                    nc.scalar.activation(
                        yt[:, half * 1024:(half + 1) * 1024], mt[:],
                        AF.Prelu, alpha=alpha, accum_out=sum_col)

            if bp == 0 and sub == 1:
                # b0's shifted half, after b1's copies have read rows 64:128
                _prep0_shift(at0)
            # defer this batch's sum(y^2) by one batch on the vector queue
            # so it never sits between a PSUM-draining pool and the PE —
            # except in the last pair, where flushing eagerly keeps the
            # final sq off the stats->all-reduce critical tail
            pending_sq.append((b, yt))
            while len(pending_sq) > (0 if bp == NB // 2 - 1 else 1):
                _emit_sq()
        if bp == 2:
            _ar_descgen()
    while pending_sq:
        _emit_sq()

    # ---- local partial stats -> remote-DMA all-reduce -> scale/shift ----
    nc.vector.tensor_reduce(loc[:, 0:1], stats[:, 0:32],
                            axis=mybir.AxisListType.X, op=ALU.add)
    nc.vector.tensor_reduce(loc[:, 1:2], stats[:, 32:48],
                            axis=mybir.AxisListType.X, op=ALU.add)
    nc.vector.tensor_copy(xbuf[:, 0:2], loc[:])

    # no_gpsimd_drain: skip the ~45us SWDGE dge_drain at block exit; the
    # kernel-tail drain picks the ring up later, off the critical path.
    with tc.tile_critical(no_gpsimd_drain=True):
        # barrier: no core fires remote writes until every peer has started
        # (prelude AllGather, normally long satisfied by now). The trigger
        # additionally inherits the deferred read of `loc`, so it waits for
        # the local stats automatically.
        nc.gpsimd.bir_kernel_barrier_wait([list(range(N_CORES))])
        nc.gpsimd.trigger_dma(count=None)
        # 7 arriving broadcasts x (16//8)=2 incs each
        nc.vector.wait_ge(rsem, 14)
        nc.vector.tensor_reduce(
            g[:, 0:1], xbuf.rearrange("p (s two) -> p two s", two=2)[:, 0:1, :],
            axis=mybir.AxisListType.X, op=ALU.add)
        nc.vector.tensor_reduce(
            g[:, 1:2], xbuf.rearrange("p (s two) -> p two s", two=2)[:, 1:2, :],
            axis=mybir.AxisListType.X, op=ALU.add)

    # mean/var/scale/shift, all [128,1] f32
    v = sp.tile([128, 8], mybir.dt.float32, name="v", tag="v")
    mean, msq_eps, vareps, std, rec, t1, s_col, t_col = (
        v[:, i:i + 1] for i in range(8))
    nc.vector.tensor_scalar(mean, g[:, 0:1], 1.0 / M_GLOBAL, None, op0=ALU.mult)
    # msq_eps = mean^2 - eps
    nc.vector.tensor_scalar(msq_eps, mean, mean, EPS, op0=ALU.mult, op1=ALU.subtract)
    # vareps = ssq/M - (mean^2 - eps) = var + eps
    nc.vector.scalar_tensor_tensor(
        vareps, g[:, 1:2], 1.0 / M_GLOBAL, msq_eps,
        op0=ALU.mult, op1=ALU.subtract)
    nc.scalar.activation(std, vareps, AF.Sqrt)
    # one Newton step: std = 0.5*(std + vareps/std)
    nc.vector.reciprocal(rec, std)
    # t1 = 0.5 * vareps / std
    nc.vector.tensor_scalar(t1, rec, vareps, 0.5, op0=ALU.mult, op1=ALU.mult)
    nc.vector.scalar_tensor_tensor(std, std, 0.5, t1,
                                   op0=ALU.mult, op1=ALU.add)
    nc.vector.reciprocal(rec, std)
    nc.vector.tensor_scalar(s_col, rec, gbt[:, 0:1], None, op0=ALU.mult)
    # t = beta - s*mean
    nc.vector.tensor_scalar(t1, mean, -1.0, None, op0=ALU.mult)
    nc.vector.scalar_tensor_tensor(
        t_col, s_col, t1, gbt[:, 1:2], op0=ALU.mult, op1=ALU.add)

    # ---- pass 2: normalize + store (f16: halves the store traffic; the
    # host widens back to f32). Two batches per output tile (fewer, bigger
    # DMAs); out-DMAs alternate across both HWDGE queues ----
    for bp in range(NB // 2):
        ot = op_pool.tile([128, 2 * LO], mybir.dt.float16, name=f"ot{bp}", tag="ot")
        for sub in range(2):
            nc.vector.tensor_scalar(
                ot[:, sub * LO:(sub + 1) * LO], y_tiles[2 * bp + sub][:],
                s_col, t_col, op0=ALU.mult, op1=ALU.add)
        eng = nc.sync if bp % 2 == 0 else nc.scalar
        eng.dma_start(
            out.rearrange("(a p) l -> p a l", p=128)[:, 2 * bp:2 * bp + 2, :],
            ot.rearrange("p (a l) -> p a l", a=2))


def _prep_weights(W: np.ndarray) -> np.ndarray:
    """Host-side: binarize conv weights and pack the 8 stationary [128,128]
    lhsT matrices (4 tap-pair layouts x even/odd batch partition layouts)."""
    bw = np.sign(W).astype(np.float32)  # [CO, CI, K]
    wts = np.zeros((128, 8, 128), dtype=np.float32)
    for j in range(4):
        # even layout: rows 0:64 direct (tap 2j), rows 64:128 shifted (tap 2j+1)
        wts[0:64, j, :] = bw[:, :, 2 * j].T
        if 2 * j + 1 < K:
            wts[64:128, j, :] = bw[:, :, 2 * j + 1].T
        # odd layout: rows 0:64 shifted (tap 2j+1), rows 64:128 direct (tap 2j)
        if 2 * j + 1 < K:
            wts[0:64, 4 + j, :] = bw[:, :, 2 * j + 1].T
        wts[64:128, 4 + j, :] = bw[:, :, 2 * j].T
    return wts.reshape(128, 8 * 128).astype(ml_dtypes.float8_e4m3fn)


_NC_CACHE = {}


def _make_in_maps(inputs):
    x = np.asarray(inputs["x"])
    wts = _prep_weights(np.asarray(inputs["W"]))
    gb = np.stack([np.asarray(inputs["gamma"], dtype=np.float32),
                   np.asarray(inputs["beta"], dtype=np.float32)], axis=1)
    in_maps = []
    for c in range(N_CORES):
        shard = np.ascontiguousarray(
            x[c * NB:(c + 1) * NB].reshape(NB * CI, L), dtype=np.float32)
        in_maps.append({"xs": shard, "wts": wts, "gb": gb.astype(np.float32)})
    return in_maps


def kernel(x, W, prelu_w, gamma, beta):
    x = np.asarray(x)
    W = np.asarray(W)
    alpha = float(np.asarray(prelu_w).reshape(-1)[0])
    gamma = np.asarray(gamma, dtype=np.float32)
    beta = np.asarray(beta, dtype=np.float32)

    assert x.shape == (N, CI, L), x.shape
    wts = _prep_weights(W)
    gb = np.stack([gamma, beta], axis=1).astype(np.float32)

    key = alpha
    if key not in _NC_CACHE:
        _NC_CACHE[key] = _build(alpha)
    nc = _NC_CACHE[key]

    in_maps = []
    for c in range(N_CORES):
        shard = np.ascontiguousarray(
            x[c * NB:(c + 1) * NB].reshape(NB * CI, L), dtype=np.float32)
        in_maps.append({"xs": shard, "wts": wts, "gb": gb})

    res = run_bass_kernel_spmd(nc, in_maps, core_ids=list(range(N_CORES)))
    outs = [res.results[c]["out"].reshape(NB, CO, LO) for c in range(N_CORES)]
    return np.concatenate(outs, axis=0).astype(np.float32)

